# revision 7
# baseline (speedup 1.0000x reference)
"""Pendulum2 DAE kernel for Trainium2 (Bass/Tile), data-parallel over 8 cores.

Closed form per sample (coords = [x0 x1 x2 x3 v0 v1 v2 v3], M0=M1=G=10):
  d0 = x0-x2, d1 = x1-x3, w0 = v0-v2, w1 = v1-v3
  s1 = x0^2+x1^2, q = x0*d0+x1*d1, r = d0^2+d1^2
  h  = v0^2+v1^2 - 10*x1, k = w0^2+w1^2
  D  = 2*s1*r - q^2
  mu1 = (2*r*h - q*k)/D, mu2 = (s1*k - q*h)/D
  out = [v0 v1 v2 v3,
         -(x0*mu1+d0*mu2), -10-(x1*mu1+d1*mu2), d0*mu2, -10+d1*mu2]

v10 design notes (v9 + trace/microbench calibration):
 - In-place output: a-outputs of sample t are written into the INPUT tile at
   group t+1 cols 0:4 (x-slots, dead by then). Bytes [16 .. 16+32*tt) of the
   in-tile then read [v_0 a_0 v_1 a_1 ...] so the out-DMA is one fully
   contiguous read per partition and the v-passthrough ACT copy (~8.5us of
   ACT across the run) disappears.
 - Microbench: split-plane (non-unit-inner) writes are 2-5 ns/el on both DVE
   and ACT -> pair layouts stay (t e)-packed exactly as v9.
 - bf16 everywhere except: the f32 input reads, lnD (bf16 ln would put ~8%
   on exp), and the two strided f32 stores of a01/a23. Emulated end-to-end
   error 1.02e-2 vs the 2e-2 gate (v9: 7.6e-3 at 9.7e-3 measured).
 - d01 in bf16 lets t0 = d01*mu2 run in the DVE 2x bf16 mode (846 vs 1235ns
   per 512-tile); D-chain bf16 saves another ~530ns/tile of DVE.
 - gpsimd offload re-tested and re-rejected: concurrent Pool+DVE aggregate
   throughput is LOWER than DVE alone (SBUF port contention).
 - vector.reciprocal / ALU divide / reciprocal_approx_fast / InstPool all
   fail to compile or mis-execute in this toolchain; 1/D stays Ln+Exp.
"""

import json

import numpy as np

from concourse import bass, bass_utils, mybir
from concourse.tile import TileContext


def _split_multi_waits(mod):
    # walrus encodes at most one sync wait per instruction; hoist extra waits
    # onto wait-only EventSemaphore nops on the same engine (in-order issue
    # preserves semantics).
    ctr = 0
    for fn in mod.get("functions", []):
        for blk in fn.get("blocks", []):
            new = []
            for inst in blk.get("instructions", []):
                si = inst.get("sync_info") or {}
                ow = si.get("on_wait") or []
                if len(ow) > 1:
                    for w in ow[:-1]:
                        ctr += 1
                        new.append(
                            {
                                "debug": inst.get("debug", 0),
                                "engine": inst["engine"],
                                "ins": [],
                                "name": f"syncsplit-{ctr}-{inst['name']}",
                                "opcode": "EventSemaphore",
                                "outs": [],
                                "sync_info": {"on_wait": [w]},
                            }
                        )
                    si = dict(si)
                    si["on_wait"] = [ow[-1]]
                    inst = dict(inst)
                    inst["sync_info"] = si
                new.append(inst)
            blk["instructions"] = new
    return mod


_ORIG_TO_JSON_BYTES = bass.Bass.to_json_bytes


def _patched_to_json_bytes(self):
    return json.dumps(_split_multi_waits(json.loads(_ORIG_TO_JSON_BYTES(self)))).encode()


bass.Bass.to_json_bytes = _patched_to_json_bytes

BS = 2_097_152
NCORES = 8
PER = BS // NCORES          # samples per core
P = 128                     # SBUF partitions
TMAX = 512
TILES = [32, 288, 512, 512, 512, 192]   # samples per partition-row per tile
assert sum(TILES) * P == PER

f32 = mybir.dt.float32
bf16 = mybir.dt.bfloat16
ALU = mybir.AluOpType
ACTF = mybir.ActivationFunctionType
SQRT2 = float(np.sqrt(2.0))

# bf16 scratch plane map (TMAX elems each):
#  0-1  w01 pk
#  2-11 square pairs [sqD(2-3), m01(4-5), sqX(6-7), sqV(8-9), sqW(10-11)]
#  fused pair-sum -> 12-16 = [r2, q, s1, h1->h, k]
#  17-18 [r2h|qh]   19-20 [qk|s1k]   21-22 [num1n|num2]
#  23 invD   24-25 [mu1n|mu2]   26 t10
#  27-30 d01/w01 quad pk   31-32 t0 pk   33 tq   34 s1r2->D   35-36 U01 pk
NB = 37
# f32 scratch: 0 lnD
NF = 1


def _build():
    nc = bass.Bass()
    coords = nc.dram_tensor("coords", [PER, 8], f32, kind="ExternalInput")
    out = nc.dram_tensor("out", [PER, 8], f32, kind="ExternalOutput")

    with TileContext(nc) as tc:
        with tc.tile_pool(name="inp", bufs=3) as inp, tc.tile_pool(
            name="sc", bufs=3
        ) as scp:
            off = 0
            for tidx, tt in enumerate(TILES):
                dram_in = coords[off : off + P * tt].rearrange("(p t) e -> p (t e)", p=P)
                dram_out = out[off : off + P * tt].rearrange("(p t) e -> p (t e)", p=P)
                off += P * tt

                in_full = inp.tile([P, TMAX * 8 + 8], f32)
                sb = scp.tile([P, NB * TMAX], bf16)
                sf = scp.tile([P, NF * TMAX], f32)
                in_t = in_full[:, : tt * 8]

                nc.sync.dma_start(out=in_t, in_=dram_in)

                iv = in_t.rearrange("p (t e) -> p t e", e=8)
                # shifted view: group t of av == group t+1 of the tile
                av = in_full[:, 8 : 8 + tt * 8].rearrange("p (t e) -> p t e", e=8)

                def bpk(a):
                    return sb[:, a * TMAX : a * TMAX + 2 * tt].rearrange(
                        "p (t e) -> p t e", e=2
                    )

                def bpl(j):
                    return sb[:, j * TMAX : j * TMAX + tt]

                def bpls(a, b):
                    return sb[:, a * TMAX : b * TMAX].rearrange(
                        "p (c t) -> p c t", t=TMAX
                    )[:, :, :tt]

                def bco(j):
                    return (
                        bpl(j).rearrange("p (o t) -> p o t", o=1).broadcast_to((P, 2, tt))
                    )

                def bc2(j):
                    return (
                        bpl(j).rearrange("p (t o) -> p t o", o=1).broadcast_to((P, tt, 2))
                    )

                V, S, G = nc.vector, nc.scalar, nc.gpsimd

                d01 = bpk(27)
                w01 = bpk(0)
                V.tensor_sub(out=d01, in0=iv[:, :, 0:2], in1=iv[:, :, 2:4])
                V.tensor_sub(out=w01, in0=iv[:, :, 4:6], in1=iv[:, :, 6:8])
                S.activation(bpk(2), d01, ACTF.Square, scale=SQRT2)              # sqD = 2d^2
                S.activation(bpk(6), iv[:, :, 0:2], ACTF.Square)                 # sqX
                S.activation(bpk(8), iv[:, :, 4:6], ACTF.Square)                 # sqV
                S.activation(bpk(10), w01, ACTF.Square)                          # sqW
                V.tensor_tensor(out=bpk(4), in0=iv[:, :, 0:2], in1=d01, op=ALU.mult)  # m01

                # all five pair-sums in one TT: bf16 planes 2-11 even vs odd
                sq5 = (
                    sb[:, 2 * TMAX : 12 * TMAX]
                    .rearrange("p (c r) -> p c r", c=5)[:, :, : 2 * tt]
                    .rearrange("p c (t e) -> p c t e", e=2)
                )
                ps = sb[:, 12 * TMAX : 17 * TMAX].rearrange("p (c t) -> p c t", c=5)[
                    :, :, :tt
                ]
                V.tensor_add(out=ps, in0=sq5[:, :, :, 0], in1=sq5[:, :, :, 1])   # [r2,q,s1,h1,k]

                # h = h1 - 10*x1
                S.activation(bpl(26), iv[:, :, 1], ACTF.Copy, scale=-10.0)       # t10
                V.tensor_add(out=bpl(15), in0=bpl(15), in1=bpl(26))              # h
                V.tensor_tensor(out=bpls(17, 19), in0=bpls(12, 14), in1=bco(15), op=ALU.mult)  # [r2h|qh]
                V.tensor_tensor(out=bpls(19, 21), in0=bpls(13, 15), in1=bco(16), op=ALU.mult)  # [qk|s1k]
                V.tensor_sub(out=bpls(21, 23), in0=bpls(19, 21), in1=bpls(17, 19))  # [num1n|num2]

                # D chain (bf16 planes; lnD stays f32)
                S.activation(bpl(33), bpl(13), ACTF.Square)                      # tq = q^2
                V.tensor_tensor(out=bpl(34), in0=bpl(14), in1=bpl(12), op=ALU.mult)  # s1*r2
                V.tensor_sub(out=bpl(34), in0=bpl(34), in1=bpl(33))              # D
                S.activation(sf[:, :tt], bpl(34), ACTF.Ln)                       # ln D (f32)
                S.activation(bpl(23), sf[:, :tt], ACTF.Exp, scale=-1.0)          # invD
                V.tensor_tensor(out=bpls(24, 26), in0=bpls(21, 23), in1=bco(23), op=ALU.mult)  # [mu1n|mu2]

                # final: t0 = d01*mu2 (bf16 pk), U01 = x01*mu1n (bf16 pk),
                # a01 = U01 - t0 -> av cols 0:2 (f32), a23 = t0 -> av cols 2:4,
                # then -10 bias on av cols 1,3
                V.tensor_tensor(out=bpk(31), in0=d01, in1=bc2(25), op=ALU.mult)  # t0
                V.tensor_tensor(out=bpk(35), in0=iv[:, :, 0:2], in1=bc2(24), op=ALU.mult)  # U01
                V.tensor_sub(out=av[:, :, 0:2], in0=bpk(35), in1=bpk(31))        # [a0, a1+10]
                S.activation(av[:, :, 2:4], bpk(31), ACTF.Copy)                  # [a2, a3+10]
                av13 = av[:, :, 1:4:2]
                S.activation(av13, av13, ACTF.Copy, bias=-10.0)                  # a1/a3 -= 10

                S.dma_start(out=dram_out, in_=in_full[:, 4 : 4 + tt * 8])
    return nc


_NC = None


def _run(coords, trace=False, **kw):
    global _NC
    if _NC is None:
        _NC = _build()
    coords = np.ascontiguousarray(coords, dtype=np.float32)
    in_maps = [
        {"coords": coords[c * PER : (c + 1) * PER]} for c in range(NCORES)
    ]
    res = bass_utils.run_bass_kernel_spmd(
        _NC, in_maps, core_ids=list(range(NCORES)), trace=trace, **kw
    )
    out = np.concatenate([res.results[c]["out"] for c in range(NCORES)], axis=0)
    return out, res


def kernel(t, coords):
    return _run(coords)[0]


# revision 20
# speedup vs baseline: 1.0372x; 1.0372x over previous
"""Pendulum2 DAE kernel for Trainium2 (Bass/Tile), data-parallel over 8 cores.

Closed form per sample (coords = [x0 x1 x2 x3 v0 v1 v2 v3], M0=M1=G=10):
  d0 = x0-x2, d1 = x1-x3, w0 = v0-v2, w1 = v1-v3
  s1 = x0^2+x1^2, q = x0*d0+x1*d1, r = d0^2+d1^2
  h  = v0^2+v1^2 - 10*x1, k = w0^2+w1^2
  D  = 2*s1*r - q^2
  mu1 = (2*r*h - q*k)/D, mu2 = (s1*k - q*h)/D
  out = [v0 v1 v2 v3,
         -(x0*mu1+d0*mu2), -10-(x1*mu1+d1*mu2), d0*mu2, -10+d1*mu2]

v10 design notes (v9 + trace/microbench calibration):
 - In-place output: a-outputs of sample t are written into the INPUT tile at
   group t+1 cols 0:4 (x-slots, dead by then). Bytes [16 .. 16+32*tt) of the
   in-tile then read [v_0 a_0 v_1 a_1 ...] so the out-DMA is one fully
   contiguous read per partition and the v-passthrough ACT copy (~8.5us of
   ACT across the run) disappears.
 - Microbench: split-plane (non-unit-inner) writes are 2-5 ns/el on both DVE
   and ACT -> pair layouts stay (t e)-packed exactly as v9.
 - bf16 everywhere except: the f32 input reads, lnD (bf16 ln would put ~8%
   on exp), and the two strided f32 stores of a01/a23. Emulated end-to-end
   error 1.02e-2 vs the 2e-2 gate (v9: 7.6e-3 at 9.7e-3 measured).
 - d01 in bf16 + bf16 D-chain + bf16 a01 keep DVE busy at ~54us and ACT at
   ~51us (balanced; ACT cannot take tensor-tensor work so no further shift).
 - out-DMA triggers issue from the ACT engine (hwDGE) so they land on their
   own hardware queue (qScalarDynamicHW) instead of interleaving with the
   in-DMA stream on qSyncDynamicHW.
 - RACE FIX: every write into the in-place output region (a01 copy, a23
   copy, -10 bias) is an ACT op, so the ACT-triggered out-DMA's single
   ACT-semaphore wait covers all of them by stream order. (A DMA descriptor
   honors ONE hw wait; _split_multi_waits hoists extras onto engine nops,
   which do NOT gate a descriptor armed elsewhere -- with a01 written by DVE
   the out-DMA sporadically read stale a-columns.) inp bufs=4 widens the
   buffer-reuse distance as insurance on the cumulative DMA-queue semaphore.
 - Measured 77.8-93us depending on chip-wide HBM contention regime (8 SPMD
   cores + co-tenants share HBM; 134MB chip traffic is the roofline).
 - gpsimd offload re-tested and re-rejected: concurrent Pool+DVE aggregate
   throughput is LOWER than DVE alone (SBUF port contention).
 - vector.reciprocal / ALU divide / reciprocal_approx_fast / InstPool all
   fail to compile or mis-execute in this toolchain; 1/D stays Ln+Exp.
"""

import json

import numpy as np

from concourse import bass, bass_utils, mybir
from concourse.tile import TileContext


def _split_multi_waits(mod):
    # walrus encodes at most one sync wait per instruction; hoist extra waits
    # onto wait-only EventSemaphore nops on the same engine (in-order issue
    # preserves semantics).
    ctr = 0
    for fn in mod.get("functions", []):
        for blk in fn.get("blocks", []):
            new = []
            for inst in blk.get("instructions", []):
                si = inst.get("sync_info") or {}
                ow = si.get("on_wait") or []
                if len(ow) > 1:
                    for w in ow[:-1]:
                        ctr += 1
                        new.append(
                            {
                                "debug": inst.get("debug", 0),
                                "engine": inst["engine"],
                                "ins": [],
                                "name": f"syncsplit-{ctr}-{inst['name']}",
                                "opcode": "EventSemaphore",
                                "outs": [],
                                "sync_info": {"on_wait": [w]},
                            }
                        )
                    si = dict(si)
                    si["on_wait"] = [ow[-1]]
                    inst = dict(inst)
                    inst["sync_info"] = si
                new.append(inst)
            blk["instructions"] = new
    return mod


_ORIG_TO_JSON_BYTES = bass.Bass.to_json_bytes


def _patched_to_json_bytes(self):
    return json.dumps(_split_multi_waits(json.loads(_ORIG_TO_JSON_BYTES(self)))).encode()


bass.Bass.to_json_bytes = _patched_to_json_bytes

BS = 2_097_152
NCORES = 8
PER = BS // NCORES          # samples per core
P = 128                     # SBUF partitions
TMAX = 512
TILES = [320, 512, 512, 512, 192]   # samples per partition-row per tile
assert sum(TILES) * P == PER

f32 = mybir.dt.float32
bf16 = mybir.dt.bfloat16
ALU = mybir.AluOpType
ACTF = mybir.ActivationFunctionType
SQRT2 = float(np.sqrt(2.0))

# bf16 scratch plane map (TMAX elems each):
#  0-1  w01 pk
#  2-11 square pairs [sqD(2-3), m01(4-5), sqX(6-7), sqV(8-9), sqW(10-11)]
#  fused pair-sum -> 12-16 = [r2, q, s1, h1->h, k]
#  17-18 [r2h|qh]   19-20 [qk|s1k]   21-22 [num1n|num2]
#  23 invD   24-25 [mu1n|mu2]   26 t10
#  27-28 d01 pk   29 tq   31-32 t0 pk   34 s1r2->D   33-34 a01 pk
#  (a01 overwrites tq/D planes -- both dead after lnD)   35-36 t0 pk (31-32 U01)
NB = 37
# f32 scratch: 0 lnD
NF = 1


def _build():
    nc = bass.Bass()
    coords = nc.dram_tensor("coords", [PER, 8], f32, kind="ExternalInput")
    out = nc.dram_tensor("out", [PER, 8], f32, kind="ExternalOutput")

    with TileContext(nc) as tc:
        with tc.tile_pool(name="inp", bufs=4) as inp, tc.tile_pool(
            name="sc", bufs=3
        ) as scp:
            off = 0
            for tidx, tt in enumerate(TILES):
                dram_in = coords[off : off + P * tt].rearrange("(p t) e -> p (t e)", p=P)
                dram_out = out[off : off + P * tt].rearrange("(p t) e -> p (t e)", p=P)
                off += P * tt

                in_full = inp.tile([P, TMAX * 8 + 8], f32)
                sb = scp.tile([P, NB * TMAX], bf16)
                sf = scp.tile([P, NF * TMAX], f32)
                in_t = in_full[:, : tt * 8]

                nc.sync.dma_start(out=in_t, in_=dram_in)

                iv = in_t.rearrange("p (t e) -> p t e", e=8)
                # shifted view: group t of av == group t+1 of the tile
                av = in_full[:, 8 : 8 + tt * 8].rearrange("p (t e) -> p t e", e=8)

                def bpk(a):
                    return sb[:, a * TMAX : a * TMAX + 2 * tt].rearrange(
                        "p (t e) -> p t e", e=2
                    )

                def bpl(j):
                    return sb[:, j * TMAX : j * TMAX + tt]

                def bpls(a, b):
                    return sb[:, a * TMAX : b * TMAX].rearrange(
                        "p (c t) -> p c t", t=TMAX
                    )[:, :, :tt]

                def bco(j):
                    return (
                        bpl(j).rearrange("p (o t) -> p o t", o=1).broadcast_to((P, 2, tt))
                    )

                def bc2(j):
                    return (
                        bpl(j).rearrange("p (t o) -> p t o", o=1).broadcast_to((P, tt, 2))
                    )

                V, S = nc.vector, nc.scalar

                d01 = bpk(27)
                w01 = bpk(0)
                V.tensor_sub(out=d01, in0=iv[:, :, 0:2], in1=iv[:, :, 2:4])
                V.tensor_sub(out=w01, in0=iv[:, :, 4:6], in1=iv[:, :, 6:8])
                S.activation(bpk(2), d01, ACTF.Square, scale=SQRT2)              # sqD = 2d^2
                S.activation(bpk(6), iv[:, :, 0:2], ACTF.Square)                 # sqX
                S.activation(bpk(8), iv[:, :, 4:6], ACTF.Square)                 # sqV
                S.activation(bpk(10), w01, ACTF.Square)                          # sqW
                V.tensor_tensor(out=bpk(4), in0=iv[:, :, 0:2], in1=d01, op=ALU.mult)  # m01

                # all five pair-sums in one TT: bf16 planes 2-11 even vs odd
                sq5 = (
                    sb[:, 2 * TMAX : 12 * TMAX]
                    .rearrange("p (c r) -> p c r", c=5)[:, :, : 2 * tt]
                    .rearrange("p c (t e) -> p c t e", e=2)
                )
                ps = sb[:, 12 * TMAX : 17 * TMAX].rearrange("p (c t) -> p c t", c=5)[
                    :, :, :tt
                ]
                V.tensor_add(out=ps, in0=sq5[:, :, :, 0], in1=sq5[:, :, :, 1])   # [r2,q,s1,h1,k]

                # h = h1 - 10*x1
                S.activation(bpl(26), iv[:, :, 1], ACTF.Copy, scale=-10.0)       # t10
                V.tensor_add(out=bpl(15), in0=bpl(15), in1=bpl(26))              # h
                # one 4-plane op: [r2h,qh,qk,s1k] = [r2,q,q,s1]*[h,h,k,k]
                # in0 uses an overlapping double-stride AP (planes 12,13,13,14);
                # all producers/consumers are DVE so in-order execution makes
                # any dep-tracker range approximation harmless.
                mn_out = sb[:, 17 * TMAX : 21 * TMAX].rearrange(
                    "p (b a t) -> p b a t", b=2, a=2
                )[:, :, :, :tt]
                mn_in0 = (
                    sb[:, 12 * TMAX : 15 * TMAX]
                    .rearrange("p (c t) -> p c t", c=3)[:, 0:2]
                    .unsqueeze(1)
                    .broadcast_to((P, 2, 2, TMAX))
                )
                ap = [list(x) for x in mn_in0.ap]
                ap[1] = [TMAX, 2]   # b-level: stride one plane (overlaps a-level)
                mn_in0.ap = mybir.VecI64Pair(ap)
                mn_in0 = mn_in0[:, :, :, :tt]
                mn_in1 = (
                    sb[:, 15 * TMAX : 17 * TMAX]
                    .rearrange("p (b t) -> p b t", b=2)
                    .unsqueeze(2)
                    .broadcast_to((P, 2, 2, TMAX))[:, :, :, :tt]
                )
                V.tensor_tensor(out=mn_out, in0=mn_in0, in1=mn_in1, op=ALU.mult)
                V.tensor_sub(out=bpls(21, 23), in0=bpls(19, 21), in1=bpls(17, 19))  # [num1n|num2]

                # D chain (bf16 planes; lnD stays f32)
                S.activation(bpl(29), bpl(13), ACTF.Square)                      # tq = q^2
                V.tensor_tensor(out=bpl(34), in0=bpl(14), in1=bpl(12), op=ALU.mult)  # s1*r2
                V.tensor_sub(out=bpl(34), in0=bpl(34), in1=bpl(29))              # D
                S.activation(sf[:, :tt], bpl(34), ACTF.Ln)                       # ln D (f32)
                S.activation(bpl(23), sf[:, :tt], ACTF.Exp, scale=-1.0)          # invD
                V.tensor_tensor(out=bpls(24, 26), in0=bpls(21, 23), in1=bco(23), op=ALU.mult)  # [mu1n|mu2]

                # final: t0 = d01*mu2 (bf16 pk), U01 = x01*mu1n (bf16 pk),
                # a01 = U01 - t0 (bf16 pk scratch). ALL writes into av then
                # happen on ACT in stream order (a01 copy, a23 copy, -10 bias)
                # and the ACT-triggered out-DMA's single ACT-semaphore wait
                # covers them unconditionally -- no reliance on multi-wait
                # splitting for the in-place output region.
                V.tensor_tensor(out=bpk(35), in0=d01, in1=bc2(25), op=ALU.mult)  # t0
                V.tensor_tensor(out=bpk(31), in0=iv[:, :, 0:2], in1=bc2(24), op=ALU.mult)  # U01
                V.tensor_sub(out=bpk(33), in0=bpk(31), in1=bpk(35))              # [a0, a1+10] bf16
                # single ACT copy of [a01 pair (pl 33), t0 pair (pl 35)] -> av cols 0:4
                fsrc = (
                    sb[:, 33 * TMAX : 37 * TMAX]
                    .rearrange("p (c u t) -> p c u t", c=2, u=2)[:, 0:1]
                    .broadcast_to((P, 2, 2, TMAX))
                )
                fap = [list(x) for x in fsrc.ap]
                fap[1] = [2, TMAX]        # t-level outer: stride 2 (pairs)
                fap[2] = [2 * TMAX, 2]    # c-level: a01 plane -> t0 plane
                fap[3] = [1, 2]           # e-level
                fsrc.ap = mybir.VecI64Pair(fap)
                fsrc = fsrc[:, :tt]
                S.activation(av[:, :, 0:4], fsrc, ACTF.Copy)                     # [a0,a1+10,a2,a3+10]
                av13 = av[:, :, 1:4:2]
                S.activation(av13, av13, ACTF.Copy, bias=-10.0)                  # a1/a3 -= 10

                S.dma_start(out=dram_out, in_=in_full[:, 4 : 4 + tt * 8])
    return nc


_NC = None


def _run(coords, trace=False, **kw):
    global _NC
    if _NC is None:
        _NC = _build()
    coords = np.ascontiguousarray(coords, dtype=np.float32)
    in_maps = [
        {"coords": coords[c * PER : (c + 1) * PER]} for c in range(NCORES)
    ]
    res = bass_utils.run_bass_kernel_spmd(
        _NC, in_maps, core_ids=list(range(NCORES)), trace=trace, **kw
    )
    out = np.concatenate([res.results[c]["out"] for c in range(NCORES)], axis=0)
    return out, res


def kernel(t, coords):
    return _run(coords)[0]


# revision 21
# speedup vs baseline: 1.0663x; 1.0281x over previous
"""Pendulum2 DAE kernel for Trainium2 (Bass/Tile), data-parallel over 8 cores.

Closed form per sample (coords = [x0 x1 x2 x3 v0 v1 v2 v3], M0=M1=G=10):
  d0 = x0-x2, d1 = x1-x3, w0 = v0-v2, w1 = v1-v3
  s1 = x0^2+x1^2, q = x0*d0+x1*d1, r = d0^2+d1^2
  h  = v0^2+v1^2 - 10*x1, k = w0^2+w1^2
  D  = 2*s1*r - q^2
  mu1 = (2*r*h - q*k)/D, mu2 = (s1*k - q*h)/D
  out = [v0 v1 v2 v3,
         -(x0*mu1+d0*mu2), -10-(x1*mu1+d1*mu2), d0*mu2, -10+d1*mu2]

v10 design notes (v9 + trace/microbench calibration):
 - In-place output: a-outputs of sample t are written into the INPUT tile at
   group t+1 cols 0:4 (x-slots, dead by then). Bytes [16 .. 16+32*tt) of the
   in-tile then read [v_0 a_0 v_1 a_1 ...] so the out-DMA is one fully
   contiguous read per partition and the v-passthrough ACT copy (~8.5us of
   ACT across the run) disappears.
 - Microbench: split-plane (non-unit-inner) writes are 2-5 ns/el on both DVE
   and ACT -> pair layouts stay (t e)-packed exactly as v9.
 - bf16 everywhere except: the f32 input reads, lnD (bf16 ln would put ~8%
   on exp), and the two strided f32 stores of a01/a23. Emulated end-to-end
   error 1.02e-2 vs the 2e-2 gate (v9: 7.6e-3 at 9.7e-3 measured).
 - d01 in bf16 + bf16 D-chain + bf16 a01 keep DVE busy at ~54us and ACT at
   ~51us (balanced; ACT cannot take tensor-tensor work so no further shift).
 - out-DMA triggers issue from the ACT engine (hwDGE) so they land on their
   own hardware queue (qScalarDynamicHW) instead of interleaving with the
   in-DMA stream on qSyncDynamicHW.
 - RACE FIX: every write into the in-place output region (a01 copy, a23
   copy, -10 bias) is an ACT op, so the ACT-triggered out-DMA's single
   ACT-semaphore wait covers all of them by stream order. (A DMA descriptor
   honors ONE hw wait; _split_multi_waits hoists extras onto engine nops,
   which do NOT gate a descriptor armed elsewhere -- with a01 written by DVE
   the out-DMA sporadically read stale a-columns.) inp bufs=4 widens the
   buffer-reuse distance as insurance on the cumulative DMA-queue semaphore.
 - v11: the two numerator multiplies fuse into one 4-plane op via an
   overlapping double-stride AP (planes 12,13,13,14 x [h,h,k,k]); the two
   final ACT stores fuse into one 4-wide copy reading [a01 pair, t0 pair]
   through a hand-built 3-level AP (mybir.VecI64Pair on AP.ap).
 - Measured 76.8-93us depending on chip-wide HBM contention regime (8 SPMD
   cores + co-tenants share HBM; 134MB chip traffic is the roofline; the
   contention also stretches compute ops via SBUF-port sharing).
 - gpsimd offload re-tested and re-rejected: concurrent Pool+DVE aggregate
   throughput is LOWER than DVE alone (SBUF port contention).
 - vector.reciprocal / ALU divide / reciprocal_approx_fast / InstPool all
   fail to compile or mis-execute in this toolchain; 1/D stays Ln+Exp.
"""

import json

import numpy as np

from concourse import bass, bass_utils, mybir
from concourse.tile import TileContext


def _split_multi_waits(mod):
    # walrus encodes at most one sync wait per instruction; hoist extra waits
    # onto wait-only EventSemaphore nops on the same engine (in-order issue
    # preserves semantics).
    ctr = 0
    for fn in mod.get("functions", []):
        for blk in fn.get("blocks", []):
            new = []
            for inst in blk.get("instructions", []):
                si = inst.get("sync_info") or {}
                ow = si.get("on_wait") or []
                if len(ow) > 1:
                    for w in ow[:-1]:
                        ctr += 1
                        new.append(
                            {
                                "debug": inst.get("debug", 0),
                                "engine": inst["engine"],
                                "ins": [],
                                "name": f"syncsplit-{ctr}-{inst['name']}",
                                "opcode": "EventSemaphore",
                                "outs": [],
                                "sync_info": {"on_wait": [w]},
                            }
                        )
                    si = dict(si)
                    si["on_wait"] = [ow[-1]]
                    inst = dict(inst)
                    inst["sync_info"] = si
                new.append(inst)
            blk["instructions"] = new
    return mod


_ORIG_TO_JSON_BYTES = bass.Bass.to_json_bytes


def _patched_to_json_bytes(self):
    return json.dumps(_split_multi_waits(json.loads(_ORIG_TO_JSON_BYTES(self)))).encode()


bass.Bass.to_json_bytes = _patched_to_json_bytes

BS = 2_097_152
NCORES = 8
PER = BS // NCORES          # samples per core
P = 128                     # SBUF partitions
TMAX = 512
TILES = [320, 512, 512, 512, 192]   # samples per partition-row per tile
assert sum(TILES) * P == PER

f32 = mybir.dt.float32
bf16 = mybir.dt.bfloat16
ALU = mybir.AluOpType
ACTF = mybir.ActivationFunctionType
SQRT2 = float(np.sqrt(2.0))

# bf16 scratch plane map (TMAX elems each):
#  0-1  w01 pk
#  2-11 square pairs [sqD(2-3), m01(4-5), sqX(6-7), sqV(8-9), sqW(10-11)]
#  fused pair-sum -> 12-16 = [r2, q, s1, h1->h, k]
#  17-18 [r2h|qh]   19-20 [qk|s1k]   21-22 [num1n|num2]
#  23 invD   24-25 [mu1n|mu2]   26 t10
#  27-28 d01 pk   29 tq   31-32 t0 pk   34 s1r2->D   33-34 a01 pk
#  (a01 overwrites tq/D planes -- both dead after lnD)   35-36 t0 pk (31-32 U01)
NB = 37
# f32 scratch: 0 lnD
NF = 1


def _build():
    nc = bass.Bass()
    coords = nc.dram_tensor("coords", [PER, 8], f32, kind="ExternalInput")
    out = nc.dram_tensor("out", [PER, 8], f32, kind="ExternalOutput")

    with TileContext(nc) as tc:
        with tc.tile_pool(name="inp", bufs=4) as inp, tc.tile_pool(
            name="sc", bufs=3
        ) as scp:
            off = 0
            for tidx, tt in enumerate(TILES):
                dram_in = coords[off : off + P * tt].rearrange("(p t) e -> p (t e)", p=P)
                dram_out = out[off : off + P * tt].rearrange("(p t) e -> p (t e)", p=P)
                off += P * tt

                in_full = inp.tile([P, TMAX * 8 + 8], f32)
                sb = scp.tile([P, NB * TMAX], bf16)
                sf = scp.tile([P, NF * TMAX], f32)
                in_t = in_full[:, : tt * 8]

                nc.sync.dma_start(out=in_t, in_=dram_in)

                iv = in_t.rearrange("p (t e) -> p t e", e=8)
                # shifted view: group t of av == group t+1 of the tile
                av = in_full[:, 8 : 8 + tt * 8].rearrange("p (t e) -> p t e", e=8)

                def bpk(a):
                    return sb[:, a * TMAX : a * TMAX + 2 * tt].rearrange(
                        "p (t e) -> p t e", e=2
                    )

                def bpl(j):
                    return sb[:, j * TMAX : j * TMAX + tt]

                def bpls(a, b):
                    return sb[:, a * TMAX : b * TMAX].rearrange(
                        "p (c t) -> p c t", t=TMAX
                    )[:, :, :tt]

                def bco(j):
                    return (
                        bpl(j).rearrange("p (o t) -> p o t", o=1).broadcast_to((P, 2, tt))
                    )

                def bc2(j):
                    return (
                        bpl(j).rearrange("p (t o) -> p t o", o=1).broadcast_to((P, tt, 2))
                    )

                V, S = nc.vector, nc.scalar

                d01 = bpk(27)
                w01 = bpk(0)
                V.tensor_sub(out=d01, in0=iv[:, :, 0:2], in1=iv[:, :, 2:4])
                V.tensor_sub(out=w01, in0=iv[:, :, 4:6], in1=iv[:, :, 6:8])
                S.activation(bpk(2), d01, ACTF.Square, scale=SQRT2)              # sqD = 2d^2
                S.activation(bpk(6), iv[:, :, 0:2], ACTF.Square)                 # sqX
                S.activation(bpk(8), iv[:, :, 4:6], ACTF.Square)                 # sqV
                S.activation(bpk(10), w01, ACTF.Square)                          # sqW
                V.tensor_tensor(out=bpk(4), in0=iv[:, :, 0:2], in1=d01, op=ALU.mult)  # m01

                # all five pair-sums in one TT: bf16 planes 2-11 even vs odd
                sq5 = (
                    sb[:, 2 * TMAX : 12 * TMAX]
                    .rearrange("p (c r) -> p c r", c=5)[:, :, : 2 * tt]
                    .rearrange("p c (t e) -> p c t e", e=2)
                )
                ps = sb[:, 12 * TMAX : 17 * TMAX].rearrange("p (c t) -> p c t", c=5)[
                    :, :, :tt
                ]
                V.tensor_add(out=ps, in0=sq5[:, :, :, 0], in1=sq5[:, :, :, 1])   # [r2,q,s1,h1,k]

                # h = h1 - 10*x1
                S.activation(bpl(26), iv[:, :, 1], ACTF.Copy, scale=-10.0)       # t10
                V.tensor_add(out=bpl(15), in0=bpl(15), in1=bpl(26))              # h
                # one 4-plane op: [r2h,qh,qk,s1k] = [r2,q,q,s1]*[h,h,k,k]
                # in0 uses an overlapping double-stride AP (planes 12,13,13,14);
                # all producers/consumers are DVE so in-order execution makes
                # any dep-tracker range approximation harmless.
                mn_out = sb[:, 17 * TMAX : 21 * TMAX].rearrange(
                    "p (b a t) -> p b a t", b=2, a=2
                )[:, :, :, :tt]
                mn_in0 = (
                    sb[:, 12 * TMAX : 15 * TMAX]
                    .rearrange("p (c t) -> p c t", c=3)[:, 0:2]
                    .unsqueeze(1)
                    .broadcast_to((P, 2, 2, TMAX))
                )
                ap = [list(x) for x in mn_in0.ap]
                ap[1] = [TMAX, 2]   # b-level: stride one plane (overlaps a-level)
                mn_in0.ap = mybir.VecI64Pair(ap)
                mn_in0 = mn_in0[:, :, :, :tt]
                mn_in1 = (
                    sb[:, 15 * TMAX : 17 * TMAX]
                    .rearrange("p (b t) -> p b t", b=2)
                    .unsqueeze(2)
                    .broadcast_to((P, 2, 2, TMAX))[:, :, :, :tt]
                )
                V.tensor_tensor(out=mn_out, in0=mn_in0, in1=mn_in1, op=ALU.mult)
                V.tensor_sub(out=bpls(21, 23), in0=bpls(19, 21), in1=bpls(17, 19))  # [num1n|num2]

                # D chain (bf16 planes; lnD stays f32)
                S.activation(bpl(29), bpl(13), ACTF.Square)                      # tq = q^2
                V.tensor_tensor(out=bpl(34), in0=bpl(14), in1=bpl(12), op=ALU.mult)  # s1*r2
                V.tensor_sub(out=bpl(34), in0=bpl(34), in1=bpl(29))              # D
                S.activation(sf[:, :tt], bpl(34), ACTF.Ln)                       # ln D (f32)
                S.activation(bpl(23), sf[:, :tt], ACTF.Exp, scale=-1.0)          # invD
                V.tensor_tensor(out=bpls(24, 26), in0=bpls(21, 23), in1=bco(23), op=ALU.mult)  # [mu1n|mu2]

                # final: t0 = d01*mu2 (bf16 pk), U01 = x01*mu1n (bf16 pk),
                # a01 = U01 - t0 (bf16 pk scratch). ALL writes into av then
                # happen on ACT in stream order (a01 copy, a23 copy, -10 bias)
                # and the ACT-triggered out-DMA's single ACT-semaphore wait
                # covers them unconditionally -- no reliance on multi-wait
                # splitting for the in-place output region.
                V.tensor_tensor(out=bpk(35), in0=d01, in1=bc2(25), op=ALU.mult)  # t0
                V.tensor_tensor(out=bpk(31), in0=iv[:, :, 0:2], in1=bc2(24), op=ALU.mult)  # U01
                V.tensor_sub(out=bpk(33), in0=bpk(31), in1=bpk(35))              # [a0, a1+10] bf16
                # single ACT copy of [a01 pair (pl 33), t0 pair (pl 35)] -> av cols 0:4
                fsrc = (
                    sb[:, 33 * TMAX : 37 * TMAX]
                    .rearrange("p (c u t) -> p c u t", c=2, u=2)[:, 0:1]
                    .broadcast_to((P, 2, 2, TMAX))
                )
                fap = [list(x) for x in fsrc.ap]
                fap[1] = [2, TMAX]        # t-level outer: stride 2 (pairs)
                fap[2] = [2 * TMAX, 2]    # c-level: a01 plane -> t0 plane
                fap[3] = [1, 2]           # e-level
                fsrc.ap = mybir.VecI64Pair(fap)
                fsrc = fsrc[:, :tt]
                S.activation(av[:, :, 0:4], fsrc, ACTF.Copy)                     # [a0,a1+10,a2,a3+10]
                av13 = av[:, :, 1:4:2]
                S.activation(av13, av13, ACTF.Copy, bias=-10.0)                  # a1/a3 -= 10

                S.dma_start(out=dram_out, in_=in_full[:, 4 : 4 + tt * 8])
    return nc


_NC = None


def _run(coords, trace=False, **kw):
    global _NC
    if _NC is None:
        _NC = _build()
    coords = np.ascontiguousarray(coords, dtype=np.float32)
    in_maps = [
        {"coords": coords[c * PER : (c + 1) * PER]} for c in range(NCORES)
    ]
    res = bass_utils.run_bass_kernel_spmd(
        _NC, in_maps, core_ids=list(range(NCORES)), trace=trace, **kw
    )
    out = np.concatenate([res.results[c]["out"] for c in range(NCORES)], axis=0)
    return out, res


def kernel(t, coords):
    return _run(coords)[0]
